# revision 1
# baseline (speedup 1.0000x reference)
"""MoE (8 experts, top-2 routing) kernel for Trainium2 — expert-parallel on 8 NeuronCores.

Strategy (per the expert-parallel sharding hint):
  * The small gate (x @ Wg + bg -> softmax -> top-2) is computed host-side
    ("replicate the small gate"); the host plays the role of the all-to-all
    dispatch: tokens are gathered per selected expert, padded to a common
    capacity C, and each NeuronCore runs one expert's MLP over its token group.
  * Per core e: y = relu(x_e @ W1[e] + b1[e]) @ W2[e], scaled by the combine
    weight per token.  b2 is folded in on the host (out += combine_w @ b2),
    which is exact regardless of b2's value.
  * The host scatter-adds the 8 weighted expert outputs back to token order.

Device kernel (per core), all matmuls in float32r (fp32 data at full PE rate):
  mm1: hT[m*128:(m+1)*128, :] = relu(W1_chunk^T @ x^T + b1)    (H on partitions)
  mm2: y[tok_tile, :]        += hT_chunk^T @ W2_chunk           (tokens on partitions)
  mm2 accumulates 8 H-chunks (one "group") in PSUM, then flushes to an SBUF
  accumulator; 4 groups cover H=4096.  Weights stream through SBUF exactly once.
"""

import numpy as np

P = 128
D = 1024
H = 4096
E = 8
TOPK = 2
DK = D // P       # 8  contraction chunks for mm1
M = H // P        # 32 H chunks
GROUP = 8         # H chunks accumulated per PSUM residency
NGROUP = M // GROUP


def _token_tiles(C):
    """Split C (multiple of 128, >=256) into chunks of 256..512 (fp32r needs
    moving dim >= 256 for full PE rate)."""
    n = C // P
    if n < 4:
        return [C]
    k, r = divmod(n, 4)
    if r == 0:
        parts = [4] * k
    elif r == 1:
        parts = [4] * (k - 1) + [3, 2]
    else:
        parts = [4] * k + [r]
    # smallest first: the first PSUM group then needs the least input data,
    # so the tensor engine starts as soon as possible after launch
    return [p * P for p in sorted(parts)]


def _build_program(C):
    import concourse.mybir as mybir
    import concourse.tile as tile
    from concourse import bacc

    f32 = mybir.dt.float32
    f32r = mybir.dt.float32r
    Relu = mybir.ActivationFunctionType.Relu
    T = C // P
    tts = _token_tiles(C)

    nc = bacc.Bacc(
        "TRN2",
        target_bir_lowering=False,
        debug=False,
        enable_asserts=True,
        num_devices=E,
    )
    xt_d = nc.dram_tensor("xt", [P, DK, C], f32r, kind="ExternalInput").ap()
    w1_d = nc.dram_tensor("w1", [P, DK, H], f32r, kind="ExternalInput").ap()
    w2_d = nc.dram_tensor("w2", [H, D], f32r, kind="ExternalInput").ap()
    b1_d = nc.dram_tensor("b1", [P, M], f32, kind="ExternalInput").ap()
    wc_d = nc.dram_tensor("wc", [P, T], f32, kind="ExternalInput").ap()
    y_d = nc.dram_tensor("y", [C, D], f32, kind="ExternalOutput").ap()

    with tile.TileContext(nc) as tc:
        with (
            tc.tile_pool(name="const", bufs=1) as const,
            tc.tile_pool(name="w1p0", bufs=4) as w1p0,
            tc.tile_pool(name="w1p", bufs=2) as w1p,
            tc.tile_pool(name="w2p", bufs=GROUP) as w2p,
            tc.tile_pool(name="htp", bufs=GROUP) as htp,
            tc.tile_pool(name="php", bufs=3, space="PSUM") as php,
            tc.tile_pool(name="pyp", bufs=2, space="PSUM") as pyp,
            tc.tile_pool(name="pwp", bufs=1, space="PSUM") as pwp,
        ):
            # Two independent HWDGE queues: weights + y on the SP queue,
            # x / biases on the ACT queue.  Within each queue, descriptors
            # drain in emission order, so emit in order of first use.
            def load_w1(pool, lo_m, n_m, tag):
                t = pool.tile([P, DK, n_m * P], f32r, tag=tag)
                nc.sync.dma_start(t[:], w1_d[:, :, lo_m * P:(lo_m + n_m) * P])
                return t

            def load_w2(m):
                t = w2p.tile([P, D], f32r, tag="w2t")
                nc.sync.dma_start(t[:], w2_d[m * P:(m + 1) * P, :])
                return t

            # Single SP HWDGE queue; emit strictly in order of first use:
            # W1(m0-1), biases, x tile 0, W1(m2-7), x tiles 1-2, W2 group 0.
            w1_g0 = [load_w1(w1p0, 0, 2, "w1h")]
            b1t = const.tile([P, M], f32)
            nc.sync.dma_start(b1t[:], b1_d[:])
            wct = const.tile([P, T], f32)
            nc.sync.dma_start(wct[:], wc_d[:])
            xt = const.tile([P, DK, C], f32r)
            for dk in range(DK):
                nc.sync.dma_start(xt[:, dk, 0:tts[0]], xt_d[:, dk, 0:tts[0]])
            off = tts[0]
            tsz1 = tts[1] if len(tts) > 1 else 0
            for half in range(1, 4):
                w1_g0.append(load_w1(w1p0, 2 * half, 2, "w1h"))
                if tsz1:
                    # interleave tile-1 x chunks between W1 half-slabs so both
                    # streams stay just ahead of the tensor engine
                    for dk in range(2 * half, 2 * half + 2):
                        nc.sync.dma_start(
                            xt[:, dk, off:off + tsz1], xt_d[:, dk, off:off + tsz1]
                        )
            if tsz1:
                for dk in (0, 1):
                    nc.sync.dma_start(
                        xt[:, dk, off:off + tsz1], xt_d[:, dk, off:off + tsz1]
                    )
                off += tsz1
            for tsz in tts[2:]:
                for dk in range(DK):
                    nc.sync.dma_start(
                        xt[:, dk, off:off + tsz], xt_d[:, dk, off:off + tsz]
                    )
                off += tsz
            w2_pre = {m: load_w2(m) for m in range(GROUP)}

            y_sb = const.tile([P, T, D], f32)

            # PE warm-up on a zeroed tile: keeps the activity monitor from
            # throttling the clock while the first operands stream in.
            warm = const.tile([P, P], f32)
            nc.any.memset(warm[:], 0.0)
            pw = pwp.tile([P, P], f32, tag="pw")
            for _ in range(12):
                nc.tensor.matmul(pw[:], warm[:], warm[:], start=True, stop=True)

            for g in range(NGROUP):
                if g == 0:
                    w1t = w1_g0
                    w1_col0 = [0, 2, 4, 6]
                    n_cols = 2
                else:
                    w1t = [
                        load_w1(w1p, g * GROUP, 4, "w1s"),
                        load_w1(w1p, g * GROUP + 4, 4, "w1s"),
                    ]
                    w1_col0 = [g * GROUP, g * GROUP + 4]
                    n_cols = 4
                w2s = [
                    w2_pre.pop(m) if m in w2_pre else load_w2(m)
                    for m in range(g * GROUP, (g + 1) * GROUP)
                ]
                hts = [
                    htp.tile([P, C], f32r, tag="ht", name=f"ht_{g}_{mi}")
                    for mi in range(GROUP)
                ]
                # mm1, token-tile-major so x/W1 chunks are needed in stream order
                off = 0
                for tsz in tts:
                    for mi in range(GROUP):
                        m = g * GROUP + mi
                        w1s = w1t[mi // n_cols]
                        c = m - w1_col0[mi // n_cols]
                        ph = php.tile([P, 512], f32, tag="ph")
                        for dk in range(DK):
                            nc.tensor.matmul(
                                ph[:, :tsz],
                                w1s[:, dk, c * P:(c + 1) * P],
                                xt[:, dk, off:off + tsz],
                                start=(dk == 0),
                                stop=(dk == DK - 1),
                            )
                        nc.scalar.activation(
                            hts[mi][:, off:off + tsz], ph[:, :tsz], Relu,
                            bias=b1t[:, m:m + 1],
                        )
                    off += tsz
                for t in range(T):
                    py = pyp.tile([P, D], f32, tag="py")
                    for mi in range(GROUP):
                        for h2 in range(2):
                            nc.tensor.matmul(
                                py[:, h2 * 512:(h2 + 1) * 512],
                                hts[mi][:, t * P:(t + 1) * P],
                                w2s[mi][:, h2 * 512:(h2 + 1) * 512],
                                start=(mi == 0),
                                stop=(mi == GROUP - 1),
                            )
                    if g == 0:
                        nc.vector.tensor_copy(y_sb[:, t, :], py[:])
                    else:
                        nc.vector.tensor_add(y_sb[:, t, :], y_sb[:, t, :], py[:])
                    if g == NGROUP - 1:
                        # final flush for this token subtile: apply the combine
                        # weight and store while later subtiles still compute
                        nc.vector.tensor_scalar_mul(
                            y_sb[:, t, :], y_sb[:, t, :], wct[:, t:t + 1]
                        )
                        nc.sync.dma_start(y_d[t * P:(t + 1) * P, :], y_sb[:, t, :])
    nc.compile()
    return nc


def _route(x, Wg, bg):
    """Host gate: softmax over experts + stable top-2 (mirrors jax.lax.top_k
    tie-breaking: lowest index first)."""
    logits = x @ Wg + bg
    mx = logits.max(axis=1, keepdims=True)
    ex = np.exp(logits - mx)
    gate = ex / ex.sum(axis=1, keepdims=True)
    top2 = np.argsort(-gate, axis=1, kind="stable")[:, :TOPK]
    return gate, top2


def _pack_core_inputs(x, gate, idx, W1e, b1e, W2e, C):
    cnt = len(idx)
    xe = np.zeros((C, D), np.float32)
    xe[:cnt] = x[idx]
    wc = np.zeros((C,), np.float32)
    wc[:cnt] = gate[idx]
    xt = np.ascontiguousarray(xe.T.reshape(DK, P, C).transpose(1, 0, 2))
    w1 = np.ascontiguousarray(W1e.reshape(DK, P, H).transpose(1, 0, 2))
    b1 = np.ascontiguousarray(b1e.reshape(M, P).T)
    wcs = np.ascontiguousarray(wc.reshape(C // P, P).T)
    return {
        "xt": xt,
        "w1": w1,
        "w2": np.ascontiguousarray(W2e),
        "b1": b1,
        "wc": wcs,
    }


def kernel(x, Wg, bg, W1, b1, W2, b2):
    from concourse.bass_utils import run_bass_kernel_spmd

    x = np.asarray(x, np.float32)
    Wg = np.asarray(Wg, np.float32)
    bg = np.asarray(bg, np.float32)
    W1 = np.asarray(W1, np.float32)
    b1 = np.asarray(b1, np.float32)
    W2 = np.asarray(W2, np.float32)
    b2 = np.asarray(b2, np.float32)
    Ttok = x.shape[0]

    gate, top2 = _route(x, Wg, bg)
    expert_idx = []
    for e in range(E):
        sel = np.nonzero((top2 == e).any(axis=1))[0]
        expert_idx.append(sel)
    max_cnt = max(len(s) for s in expert_idx)
    C = max(256, -(-max_cnt // P) * P)

    nc = _build_program(C)
    in_maps = [
        _pack_core_inputs(x, gate[:, e], expert_idx[e], W1[e], b1[e], W2[e], C)
        for e in range(E)
    ]
    results = run_bass_kernel_spmd(nc, in_maps, core_ids=list(range(E))).results

    out = np.zeros((Ttok, D), np.float32)
    for e in range(E):
        idx = expert_idx[e]
        out[idx] += results[e]["y"][: len(idx)]
    # b2 contribution, folded on the host (exact: y*(w) device + w*b2 here)
    mask = np.zeros((Ttok, E), np.float32)
    np.put_along_axis(mask, top2, 1.0, axis=1)
    out += (gate * mask) @ b2
    return out



# revision 5
# speedup vs baseline: 1.2925x; 1.2925x over previous
"""MoE (8 experts, top-2 routing) kernel for Trainium2 — 8 NeuronCores.

Sharding: expert-pair parallel with H-split.  Experts are paired
big-count-with-small-count; pair p is served by cores 2p (H-half 0) and
2p+1 (H-half 1).  Each core runs BOTH experts of its pair over the
pair's full token groups, but only its half of the hidden dimension
(H/2 = 2048), producing partial y that the host sums.  This balances
tokens across cores (~2111/core vs 2*1152 for naive expert-parallel)
while keeping weight traffic identical (each core holds half of two
experts = one expert's worth of weights).

The small gate runs host-side; the host gathers tokens per expert,
pads to the common SPMD capacities (Ca = max big-expert count, Cb =
max small count), and scatter-adds the 16 partial outputs (2 cores x
2 experts per token) back to token order.  b2 is folded on the host
(out += combine_w @ b2, exact for any b2).

Device kernel (per core), all matmuls bf16 (fp32 PSUM accumulation):
  for e in (a, b):
    mm1 (tile-major): hT[mi][:, tile] = relu(W1_mi^T @ x^T + b1)  bf16
    mm2 (token-tile): py[t] = sum_mi hT[mi][:,t]^T @ W2_mi   (one PSUM
        residency accumulates the whole H/2 contraction)
    flush: y[t] = py[t] * wc[t]  on the scalar engine (per-partition
        combine-weight scale), DMA out from SBUF on the ACT queue.
Weights stream through SBUF exactly once; W1 of the running expert is
fully resident (32 KB/partition bf16)."""

import numpy as np

P = 128
D = 1024
H = 4096
HH = H // 2       # per-core hidden half
E = 8
TOPK = 2
DK = D // P       # 8  contraction chunks for mm1
MH = HH // P      # 16 hidden chunks per core per expert


def _mm1_tiles(C, head128):
    """Split C tokens into matmul moving-dim chunks (<=512 for one PSUM
    bank).  If head128, carve a 128-token head tile so the very first
    PSUM group needs minimal input data (fast launch)."""
    tiles = []
    rem = C
    if head128 and C > 512:
        tiles.append(min(128, C))
        rem = C - tiles[0]
    n = -(-rem // 512)
    if n:
        base, r = divmod(rem, n)
        tiles += [base + (1 if i < r else 0) for i in range(n)]
    return tiles


def _build_program(Ca, Cb):
    import concourse.mybir as mybir
    import concourse.tile as tile
    from concourse import bacc

    f32 = mybir.dt.float32
    bf16 = mybir.dt.bfloat16
    Relu = mybir.ActivationFunctionType.Relu
    Copy = mybir.ActivationFunctionType.Copy
    Ta = -(-Ca // P)
    Tb = -(-Cb // P)
    tts_a = _mm1_tiles(Ca, head128=True)
    tts_b = _mm1_tiles(Cb, head128=False)

    nc = bacc.Bacc(
        "TRN2",
        target_bir_lowering=False,
        debug=False,
        enable_asserts=True,
        num_devices=E,
    )
    xa_d = nc.dram_tensor("xa", [P, DK, Ca], bf16, kind="ExternalInput").ap()
    xb_d = nc.dram_tensor("xb", [P, DK, Cb], bf16, kind="ExternalInput").ap()
    w1a_d = nc.dram_tensor("w1a", [P, DK, HH], bf16, kind="ExternalInput").ap()
    w1b_d = nc.dram_tensor("w1b", [P, DK, HH], bf16, kind="ExternalInput").ap()
    w2a_d = nc.dram_tensor("w2a", [HH, D], bf16, kind="ExternalInput").ap()
    w2b_d = nc.dram_tensor("w2b", [HH, D], bf16, kind="ExternalInput").ap()
    b1a_d = nc.dram_tensor("b1a", [P, MH], f32, kind="ExternalInput").ap()
    b1b_d = nc.dram_tensor("b1b", [P, MH], f32, kind="ExternalInput").ap()
    wca_d = nc.dram_tensor("wca", [P, Ta], f32, kind="ExternalInput").ap()
    wcb_d = nc.dram_tensor("wcb", [P, Tb], f32, kind="ExternalInput").ap()
    ya_d = nc.dram_tensor("ya", [Ca, D], f32, kind="ExternalOutput").ap()
    yb_d = nc.dram_tensor("yb", [Cb, D], f32, kind="ExternalOutput").ap()

    with tile.TileContext(nc) as tc:
        with (
            tc.tile_pool(name="const", bufs=1) as const,
            tc.tile_pool(name="w1p", bufs=2) as w1p,
            tc.tile_pool(name="w2p", bufs=24) as w2p,
            tc.tile_pool(name="htp", bufs=MH) as htp,
            tc.tile_pool(name="ysp", bufs=3) as ysp,
            tc.tile_pool(name="php", bufs=3, space="PSUM") as php,
            tc.tile_pool(name="pyp", bufs=2, space="PSUM") as pyp,
            tc.tile_pool(name="pwp", bufs=1, space="PSUM") as pwp,
        ):
            # ---- SBUF destination tiles -------------------------------
            # W1 of each expert lives fully in SBUF while its mm1 runs
            # (16 KB/partition bf16); per-m-chunk DMAs so the tensor
            # engine can chase the stream from the first chunk.
            w1t = {}
            xa = const.tile([P, DK, Ca], bf16)
            xb = const.tile([P, DK, Cb], bf16)
            b1a = const.tile([P, MH], f32)
            b1b = const.tile([P, MH], f32)
            wca = const.tile([P, Ta], f32)
            wcb = const.tile([P, Tb], f32)

            def load_w1(which, w1_d):
                t = w1p.tile([P, DK, HH], bf16, tag="w1", name=f"w1_{which}")
                for mi in range(MH):
                    nc.sync.dma_start(
                        t[:, :, mi * P:(mi + 1) * P],
                        w1_d[:, :, mi * P:(mi + 1) * P],
                    )
                w1t[which] = t

            def load_x(xt, x_d, tts):
                off = 0
                for tsz in tts:
                    nc.sync.dma_start(
                        xt[:, :, off:off + tsz], x_d[:, :, off:off + tsz]
                    )
                    off += tsz

            def load_w2(w2_d, mi, name):
                t = w2p.tile([P, D], bf16, tag="w2", name=name)
                nc.sync.dma_start(t[:], w2_d[mi * P:(mi + 1) * P, :])
                return t

            # ---- DMA emission order (single in-order SP queue) --------
            # expert a: x tile0, w1a chunk-by-chunk (interleaved with the
            # larger x tiles), biases, then W2a; expert b's inputs follow
            # and stream during expert a's compute.
            nc.sync.dma_start(xa[:, :, 0:tts_a[0]], xa_d[:, :, 0:tts_a[0]])
            ta = w1p.tile([P, DK, HH], bf16, tag="w1", name="w1_a")
            w1t["a"] = ta
            for mi in range(4):
                nc.sync.dma_start(
                    ta[:, :, mi * P:(mi + 1) * P], w1a_d[:, :, mi * P:(mi + 1) * P]
                )
            nc.sync.dma_start(b1a[:], b1a_d[:])
            nc.sync.dma_start(wca[:], wca_d[:])
            off = tts_a[0]
            if len(tts_a) > 1:
                nc.sync.dma_start(
                    xa[:, :, off:off + tts_a[1]], xa_d[:, :, off:off + tts_a[1]]
                )
                off += tts_a[1]
            for mi in range(4, MH):
                nc.sync.dma_start(
                    ta[:, :, mi * P:(mi + 1) * P], w1a_d[:, :, mi * P:(mi + 1) * P]
                )
            for tsz in tts_a[2:]:
                nc.sync.dma_start(xa[:, :, off:off + tsz], xa_d[:, :, off:off + tsz])
                off += tsz
            w2a = [load_w2(w2a_d, mi, f"w2a_{mi}") for mi in range(MH)]
            nc.sync.dma_start(b1b[:], b1b_d[:])
            nc.sync.dma_start(wcb[:], wcb_d[:])
            load_x(xb, xb_d, tts_b)
            load_w1("b", w1b_d)
            w2b = [load_w2(w2b_d, mi, f"w2b_{mi}") for mi in range(MH)]

            # ---- PE warm-up on a zeroed tile: ramps the DVFS while the
            # first operands stream in.
            warm = const.tile([P, P], bf16)
            nc.any.memset(warm[:], 0.0)
            pw = pwp.tile([P, P], f32, tag="pw")
            for _ in range(12):
                nc.tensor.matmul(pw[:], warm[:], warm[:], start=True, stop=True)

            # ---- per-expert compute -----------------------------------
            def expert(which, C, tts, T, b1t, wct, w2s, y_d):
                w1s = w1t[which]
                xt = xa if which == "a" else xb
                hts = [
                    htp.tile([P, Ca], bf16, tag="ht", name=f"ht_{which}_{mi}")
                    for mi in range(MH)
                ]
                # mm1, tile-major: x tile0 + first W1 chunk suffice to start
                off = 0
                for tsz in tts:
                    for mi in range(MH):
                        ph = php.tile([P, 512], f32, tag="ph")
                        for dk in range(DK):
                            nc.tensor.matmul(
                                ph[:, :tsz],
                                w1s[:, dk, mi * P:(mi + 1) * P],
                                xt[:, dk, off:off + tsz],
                                start=(dk == 0),
                                stop=(dk == DK - 1),
                            )
                        nc.scalar.activation(
                            hts[mi][:, off:off + tsz], ph[:, :tsz], Relu,
                            bias=b1t[:, mi:mi + 1],
                        )
                    off += tsz
                # mm2: whole H/2 contraction in one PSUM residency per
                # 128-token tile, then scale-by-combine-weight flush.
                for t in range(T):
                    np_ = min(P, C - t * P)
                    py = pyp.tile([P, D], f32, tag="py")
                    for mi in range(MH):
                        for h2 in range(2):
                            nc.tensor.matmul(
                                py[:np_, h2 * 512:(h2 + 1) * 512],
                                hts[mi][:, t * P:t * P + np_],
                                w2s[mi][:, h2 * 512:(h2 + 1) * 512],
                                start=(mi == 0),
                                stop=(mi == MH - 1),
                            )
                    ys = ysp.tile([P, D], f32, tag="ys")
                    last = which == "b" and t == T - 1
                    if last:
                        # split the final flush so the store starts earlier
                        for h2 in range(2):
                            nc.scalar.activation(
                                ys[:np_, h2 * 512:(h2 + 1) * 512],
                                py[:np_, h2 * 512:(h2 + 1) * 512],
                                Copy, scale=wct[:np_, t:t + 1],
                            )
                            nc.scalar.dma_start(
                                y_d[t * P:t * P + np_, h2 * 512:(h2 + 1) * 512],
                                ys[:np_, h2 * 512:(h2 + 1) * 512],
                            )
                    else:
                        nc.scalar.activation(
                            ys[:np_], py[:np_], Copy, scale=wct[:np_, t:t + 1]
                        )
                        nc.scalar.dma_start(y_d[t * P:t * P + np_, :], ys[:np_])

            expert("a", Ca, tts_a, Ta, b1a, wca, w2a, ya_d)
            expert("b", Cb, tts_b, Tb, b1b, wcb, w2b, yb_d)
    nc.compile()
    return nc


def _route(x, Wg, bg):
    """Host gate: softmax over experts + stable top-2 (mirrors
    jax.lax.top_k tie-breaking: lowest index first)."""
    logits = x @ Wg + bg
    mx = logits.max(axis=1, keepdims=True)
    ex = np.exp(logits - mx)
    gate = ex / ex.sum(axis=1, keepdims=True)
    top2 = np.argsort(-gate, axis=1, kind="stable")[:, :TOPK]
    return gate, top2


def _pack_x(x, idx, C, bf16):
    """[C, D] gathered tokens -> [P, DK, C] bf16 (D on partitions)."""
    xe = np.zeros((C, D), np.float32)
    xe[: len(idx)] = x[idx]
    xt = xe.T.reshape(DK, P, C).transpose(1, 0, 2)
    return np.ascontiguousarray(xt).astype(bf16)


def _pack_wc(gate_col, idx, C):
    """Combine weights for one expert -> [P, ceil(C/P)] (token-chunked,
    zero-padded past the token count)."""
    T = -(-C // P)
    wc = np.zeros((T * P,), np.float32)
    wc[: len(idx)] = gate_col[idx]
    return np.ascontiguousarray(wc.reshape(T, P).T)


def kernel(x, Wg, bg, W1, b1, W2, b2):
    import ml_dtypes
    from concourse.bass_utils import run_bass_kernel_spmd

    bf16 = ml_dtypes.bfloat16
    x = np.asarray(x, np.float32)
    Wg = np.asarray(Wg, np.float32)
    bg = np.asarray(bg, np.float32)
    W1 = np.asarray(W1, np.float32)
    b1 = np.asarray(b1, np.float32)
    W2 = np.asarray(W2, np.float32)
    b2 = np.asarray(b2, np.float32)
    Ttok = x.shape[0]

    gate, top2 = _route(x, Wg, bg)
    expert_idx = [
        np.nonzero((top2 == e).any(axis=1))[0] for e in range(E)
    ]
    cnts = np.array([len(s) for s in expert_idx])
    order = np.argsort(-cnts, kind="stable")
    bigs = order[:4]
    smalls = order[4:][::-1]          # pair i-th largest with i-th smallest
    # exact SPMD capacities (mm1 cost scales with C; only the wc layout
    # and mm2 tile count are 128-granular)
    Ca = max(P * 2, int(cnts[bigs].max()))
    Cb = max(P * 2, int(cnts[smalls].max()))

    nc = _build_program(Ca, Cb)

    in_maps = []
    for p in range(4):
        ea, eb = int(bigs[p]), int(smalls[p])
        ia, ib = expert_idx[ea], expert_idx[eb]
        xa = _pack_x(x, ia, Ca, bf16)
        xb = _pack_x(x, ib, Cb, bf16)
        wca = _pack_wc(gate[:, ea], ia, Ca)
        wcb = _pack_wc(gate[:, eb], ib, Cb)
        for half in range(2):
            hs = slice(half * HH, (half + 1) * HH)
            w1a = np.ascontiguousarray(
                W1[ea][:, hs].reshape(DK, P, HH).transpose(1, 0, 2)
            ).astype(bf16)
            w1b = np.ascontiguousarray(
                W1[eb][:, hs].reshape(DK, P, HH).transpose(1, 0, 2)
            ).astype(bf16)
            in_maps.append({
                "xa": xa, "xb": xb,
                "w1a": w1a, "w1b": w1b,
                "w2a": np.ascontiguousarray(W2[ea][hs, :]).astype(bf16),
                "w2b": np.ascontiguousarray(W2[eb][hs, :]).astype(bf16),
                "b1a": np.ascontiguousarray(b1[ea][hs].reshape(MH, P).T),
                "b1b": np.ascontiguousarray(b1[eb][hs].reshape(MH, P).T),
                "wca": wca, "wcb": wcb,
            })

    results = run_bass_kernel_spmd(nc, in_maps, core_ids=list(range(E))).results

    out = np.zeros((Ttok, D), np.float32)
    for p in range(4):
        ea, eb = int(bigs[p]), int(smalls[p])
        ia, ib = expert_idx[ea], expert_idx[eb]
        for half in range(2):
            r = results[2 * p + half]
            out[ia] += r["ya"][: len(ia)]
            out[ib] += r["yb"][: len(ib)]
    # b2 contribution, folded on the host (exact for any b2)
    mask = np.zeros((Ttok, E), np.float32)
    np.put_along_axis(mask, top2, 1.0, axis=1)
    out += (gate * mask) @ b2
    return out


# revision 13
# speedup vs baseline: 1.3607x; 1.0528x over previous
"""MoE (8 experts, top-2 routing) kernel for Trainium2 — 8 NeuronCores.

Sharding: expert-pair parallel with H-split.  Experts are paired
big-count-with-small-count; pair p is served by cores 2p (H-half 0) and
2p+1 (H-half 1).  Each core runs BOTH experts of its pair over the
pair's full token groups, but only its half of the hidden dimension
(H/2 = 2048), producing partial y that the host sums.  This balances
tokens across cores (~2111/core vs 2*1152 for naive expert-parallel)
while keeping weight traffic identical (each core holds half of two
experts = one expert's worth of weights).

The small gate runs host-side; the host gathers tokens per expert,
pads to the common SPMD capacities (Ca = max big-expert count, Cb =
max small count), and scatter-adds the 16 partial outputs (2 cores x
2 experts per token) back to token order.  b2 is folded on the host
(out += combine_w @ b2, exact for any b2).

Device kernel (per core), all matmuls bf16 (fp32 PSUM accumulation):
  for e in (a, b):
    mm1 (tile-major): hT[mi][:, tile] = relu(W1_mi^T @ x^T + b1)  bf16
    mm2 (token-tile): py[t] = sum_mi hT[mi][:,t]^T @ W2_mi   (one PSUM
        residency accumulates the whole H/2 contraction)
    flush: y[t] = py[t] * wc[t]  on the scalar engine (per-partition
        combine-weight scale), DMA out from SBUF on the ACT queue.
Weights stream through SBUF exactly once; W1 of the running expert is
fully resident (32 KB/partition bf16)."""

import numpy as np

P = 128
D = 1024
H = 4096
HH = H // 2       # per-core hidden half
E = 8
TOPK = 2
DK = D // P       # 8  contraction chunks for mm1
MH = HH // P      # 16 hidden chunks per core per expert


def _mm1_tiles(C, head256):
    """Split C tokens into matmul moving-dim chunks (<=512 for one PSUM
    bank, >=256 so the bf16 matmul is not LDWEIGHTS-paced).  If head256,
    carve a 256-token head tile so the very first PSUM group needs
    minimal input data (fast launch) without adding a tile."""
    tiles = []
    rem = C
    if head256 and C > 768 and -(-(C - 256) // 512) < -(-C // 512):
        tiles.append(256)
        rem = C - 256
    n = max(1, -(-rem // 512))
    base, r = divmod(rem, n)
    tiles += [base + (1 if i < r else 0) for i in range(n)]
    return tiles


def _build_program(Ca, Cb):
    import concourse.mybir as mybir
    import concourse.tile as tile
    from concourse import bacc

    f32 = mybir.dt.float32
    bf16 = mybir.dt.bfloat16
    Relu = mybir.ActivationFunctionType.Relu
    Copy = mybir.ActivationFunctionType.Copy
    Ta = -(-Ca // P)
    Tb = -(-Cb // P)
    tts_a = _mm1_tiles(Ca, head256=True)
    tts_b = _mm1_tiles(Cb, head256=False)

    nc = bacc.Bacc(
        "TRN2",
        target_bir_lowering=False,
        debug=False,
        enable_asserts=True,
        num_devices=E,
    )
    xa_d = nc.dram_tensor("xa", [P, DK, Ca], bf16, kind="ExternalInput").ap()
    xb_d = nc.dram_tensor("xb", [P, DK, Cb], bf16, kind="ExternalInput").ap()
    # W1 chunk-major: [P, MH, DK, P] so each per-chunk DMA moves a
    # contiguous 2 KB per partition (full-rate DMA bursts)
    w1a_d = nc.dram_tensor("w1a", [P, MH, DK, P], bf16, kind="ExternalInput").ap()
    w1b_d = nc.dram_tensor("w1b", [P, MH, DK, P], bf16, kind="ExternalInput").ap()
    w2a_d = nc.dram_tensor("w2a", [HH, D], bf16, kind="ExternalInput").ap()
    w2b_d = nc.dram_tensor("w2b", [HH, D], bf16, kind="ExternalInput").ap()
    b1a_d = nc.dram_tensor("b1a", [P, MH], f32, kind="ExternalInput").ap()
    b1b_d = nc.dram_tensor("b1b", [P, MH], f32, kind="ExternalInput").ap()
    wca_d = nc.dram_tensor("wca", [P, Ta], f32, kind="ExternalInput").ap()
    wcb_d = nc.dram_tensor("wcb", [P, Tb], f32, kind="ExternalInput").ap()
    ya_d = nc.dram_tensor("ya", [Ca, D], f32, kind="ExternalOutput").ap()
    yb_d = nc.dram_tensor("yb", [Cb, D], f32, kind="ExternalOutput").ap()

    with tile.TileContext(nc) as tc:
        with (
            tc.tile_pool(name="const", bufs=1) as const,
            tc.tile_pool(name="w1p", bufs=2) as w1p,
            tc.tile_pool(name="w2p", bufs=24) as w2p,
            tc.tile_pool(name="htp", bufs=MH) as htp,
            tc.tile_pool(name="ysp", bufs=3) as ysp,
            tc.tile_pool(name="php", bufs=3, space="PSUM") as php,
            tc.tile_pool(name="pyp", bufs=2, space="PSUM") as pyp,
            tc.tile_pool(name="pwp", bufs=1, space="PSUM") as pwp,
        ):
            # ---- SBUF destination tiles -------------------------------
            # W1 of each expert lives fully in SBUF while its mm1 runs
            # (16 KB/partition bf16); per-m-chunk DMAs so the tensor
            # engine can chase the stream from the first chunk.
            w1t = {}
            xa = const.tile([P, DK, Ca], bf16)
            xb = const.tile([P, DK, Cb], bf16)
            b1a = const.tile([P, MH], f32)
            b1b = const.tile([P, MH], f32)
            wca = const.tile([P, Ta], f32)
            wcb = const.tile([P, Tb], f32)

            def load_w1(which, w1_d):
                t = w1p.tile([P, MH, DK, P], bf16, tag="w1", name=f"w1_{which}")
                for mi in range(MH):
                    nc.sync.dma_start(t[:, mi], w1_d[:, mi])
                w1t[which] = t

            def load_x(xt, x_d, tts):
                off = 0
                for tsz in tts:
                    nc.sync.dma_start(
                        xt[:, :, off:off + tsz], x_d[:, :, off:off + tsz]
                    )
                    off += tsz

            def load_w2(w2_d, mi, name):
                t = w2p.tile([P, D], bf16, tag="w2", name=name)
                nc.sync.dma_start(t[:], w2_d[mi * P:(mi + 1) * P, :])
                return t

            # ---- DMA emission order (single in-order SP queue) --------
            # expert a: x tile0, w1a chunk-by-chunk (interleaved with the
            # larger x tiles), biases, then W2a; expert b's inputs follow
            # and stream during expert a's compute.
            nc.sync.dma_start(xa[:, :, 0:tts_a[0]], xa_d[:, :, 0:tts_a[0]])
            ta = w1p.tile([P, MH, DK, P], bf16, tag="w1", name="w1_a")
            w1t["a"] = ta
            for mi in range(4):
                nc.sync.dma_start(ta[:, mi], w1a_d[:, mi])
            nc.sync.dma_start(b1a[:], b1a_d[:])
            nc.sync.dma_start(wca[:], wca_d[:])
            off = tts_a[0]
            if len(tts_a) > 1:
                nc.sync.dma_start(
                    xa[:, :, off:off + tts_a[1]], xa_d[:, :, off:off + tts_a[1]]
                )
                off += tts_a[1]
            for mi in range(4, MH):
                nc.sync.dma_start(ta[:, mi], w1a_d[:, mi])
            for tsz in tts_a[2:]:
                nc.sync.dma_start(xa[:, :, off:off + tsz], xa_d[:, :, off:off + tsz])
                off += tsz
            w2a = [load_w2(w2a_d, mi, f"w2a_{mi}") for mi in range(MH)]
            nc.sync.dma_start(b1b[:], b1b_d[:])
            nc.sync.dma_start(wcb[:], wcb_d[:])
            load_x(xb, xb_d, tts_b)
            load_w1("b", w1b_d)
            w2b = [load_w2(w2b_d, mi, f"w2b_{mi}") for mi in range(MH)]

            # ---- PE warm-up on a zeroed tile: ramps the DVFS while the
            # first operands stream in.
            warm = const.tile([P, P], bf16)
            nc.any.memset(warm[:], 0.0)
            pw = pwp.tile([P, P], f32, tag="pw")
            for _ in range(12):
                nc.tensor.matmul(pw[:], warm[:], warm[:], start=True, stop=True)

            # ---- per-expert compute -----------------------------------
            def expert(which, C, tts, T, b1t, wct, w2s, y_d):
                w1s = w1t[which]
                xt = xa if which == "a" else xb
                hts = [
                    htp.tile([P, Ca], bf16, tag="ht", name=f"ht_{which}_{mi}")
                    for mi in range(MH)
                ]
                # mm1, tile-major: x tile0 + first W1 chunk suffice to start
                off = 0
                for tsz in tts:
                    for mi in range(MH):
                        ph = php.tile([P, 512], f32, tag="ph")
                        for dk in range(DK):
                            nc.tensor.matmul(
                                ph[:, :tsz],
                                w1s[:, mi, dk, :],
                                xt[:, dk, off:off + tsz],
                                start=(dk == 0),
                                stop=(dk == DK - 1),
                            )
                        nc.scalar.activation(
                            hts[mi][:, off:off + tsz], ph[:, :tsz], Relu,
                            bias=b1t[:, mi:mi + 1],
                        )
                    off += tsz
                # mm2: whole H/2 contraction in one PSUM residency per
                # 128-token tile, then scale-by-combine-weight flush.
                for t in range(T):
                    np_ = min(P, C - t * P)
                    py = pyp.tile([P, D], f32, tag="py")
                    for mi in range(MH):
                        for h2 in range(2):
                            nc.tensor.matmul(
                                py[:np_, h2 * 512:(h2 + 1) * 512],
                                hts[mi][:, t * P:t * P + np_],
                                w2s[mi][:, h2 * 512:(h2 + 1) * 512],
                                start=(mi == 0),
                                stop=(mi == MH - 1),
                            )
                    ys = ysp.tile([P, D], f32, tag="ys")
                    last = which == "b" and t == T - 1
                    if last:
                        # final flush: scalar and vector engines each take
                        # half so the last store leaves ASAP
                        nc.scalar.activation(
                            ys[:np_, 0:512], py[:np_, 0:512],
                            Copy, scale=wct[:np_, t:t + 1],
                        )
                        nc.vector.tensor_scalar_mul(
                            ys[:np_, 512:1024], py[:np_, 512:1024],
                            wct[:np_, t:t + 1],
                        )
                        nc.sync.dma_start(
                            y_d[t * P:t * P + np_, 0:512], ys[:np_, 0:512]
                        )
                        nc.sync.dma_start(
                            y_d[t * P:t * P + np_, 512:1024], ys[:np_, 512:1024]
                        )
                    else:
                        nc.scalar.activation(
                            ys[:np_], py[:np_], Copy, scale=wct[:np_, t:t + 1]
                        )
                        # stores ride the otherwise-idle SP queue
                        nc.sync.dma_start(y_d[t * P:t * P + np_, :], ys[:np_])

            expert("a", Ca, tts_a, Ta, b1a, wca, w2a, ya_d)
            expert("b", Cb, tts_b, Tb, b1b, wcb, w2b, yb_d)
    nc.compile()
    return nc


def _route(x, Wg, bg):
    """Host gate: softmax over experts + stable top-2 (mirrors
    jax.lax.top_k tie-breaking: lowest index first)."""
    logits = x @ Wg + bg
    mx = logits.max(axis=1, keepdims=True)
    ex = np.exp(logits - mx)
    gate = ex / ex.sum(axis=1, keepdims=True)
    top2 = np.argsort(-gate, axis=1, kind="stable")[:, :TOPK]
    return gate, top2


def _pack_x(x, idx, C, bf16):
    """[C, D] gathered tokens -> [P, DK, C] bf16 (D on partitions)."""
    xe = np.zeros((C, D), np.float32)
    xe[: len(idx)] = x[idx]
    xt = xe.T.reshape(DK, P, C).transpose(1, 0, 2)
    return np.ascontiguousarray(xt).astype(bf16)


def _pack_wc(gate_col, idx, C):
    """Combine weights for one expert -> [P, ceil(C/P)] (token-chunked,
    zero-padded past the token count)."""
    T = -(-C // P)
    wc = np.zeros((T * P,), np.float32)
    wc[: len(idx)] = gate_col[idx]
    return np.ascontiguousarray(wc.reshape(T, P).T)


def kernel(x, Wg, bg, W1, b1, W2, b2):
    import ml_dtypes
    from concourse.bass_utils import run_bass_kernel_spmd

    bf16 = ml_dtypes.bfloat16
    x = np.asarray(x, np.float32)
    Wg = np.asarray(Wg, np.float32)
    bg = np.asarray(bg, np.float32)
    W1 = np.asarray(W1, np.float32)
    b1 = np.asarray(b1, np.float32)
    W2 = np.asarray(W2, np.float32)
    b2 = np.asarray(b2, np.float32)
    Ttok = x.shape[0]

    gate, top2 = _route(x, Wg, bg)
    expert_idx = [
        np.nonzero((top2 == e).any(axis=1))[0] for e in range(E)
    ]
    cnts = np.array([len(s) for s in expert_idx])
    order = np.argsort(-cnts, kind="stable")
    bigs = order[:4]
    smalls = order[4:][::-1]          # pair i-th largest with i-th smallest
    # exact SPMD capacities (mm1 cost scales with C; only the wc layout
    # and mm2 tile count are 128-granular)
    Ca = max(P * 2, int(cnts[bigs].max()))
    Cb = max(P * 2, int(cnts[smalls].max()))

    nc = _build_program(Ca, Cb)

    in_maps = []
    for p in range(4):
        ea, eb = int(bigs[p]), int(smalls[p])
        ia, ib = expert_idx[ea], expert_idx[eb]
        xa = _pack_x(x, ia, Ca, bf16)
        xb = _pack_x(x, ib, Cb, bf16)
        wca = _pack_wc(gate[:, ea], ia, Ca)
        wcb = _pack_wc(gate[:, eb], ib, Cb)
        for half in range(2):
            hs = slice(half * HH, (half + 1) * HH)
            w1a = np.ascontiguousarray(
                W1[ea][:, hs].reshape(DK, P, MH, P).transpose(1, 2, 0, 3)
            ).astype(bf16)
            w1b = np.ascontiguousarray(
                W1[eb][:, hs].reshape(DK, P, MH, P).transpose(1, 2, 0, 3)
            ).astype(bf16)
            in_maps.append({
                "xa": xa, "xb": xb,
                "w1a": w1a, "w1b": w1b,
                "w2a": np.ascontiguousarray(W2[ea][hs, :]).astype(bf16),
                "w2b": np.ascontiguousarray(W2[eb][hs, :]).astype(bf16),
                "b1a": np.ascontiguousarray(b1[ea][hs].reshape(MH, P).T),
                "b1b": np.ascontiguousarray(b1[eb][hs].reshape(MH, P).T),
                "wca": wca, "wcb": wcb,
            })

    results = run_bass_kernel_spmd(nc, in_maps, core_ids=list(range(E))).results

    out = np.zeros((Ttok, D), np.float32)
    for p in range(4):
        ea, eb = int(bigs[p]), int(smalls[p])
        ia, ib = expert_idx[ea], expert_idx[eb]
        for half in range(2):
            r = results[2 * p + half]
            out[ia] += r["ya"][: len(ia)]
            out[ib] += r["yb"][: len(ib)]
    # b2 contribution, folded on the host (exact for any b2)
    mask = np.zeros((Ttok, E), np.float32)
    np.put_along_axis(mask, top2, 1.0, axis=1)
    out += (gate * mask) @ b2
    return out


# revision 21
# speedup vs baseline: 1.3697x; 1.0066x over previous
"""MoE (8 experts, top-2 routing) kernel for Trainium2 — 8 NeuronCores.

Sharding: expert-pair parallel with H-split.  Experts are paired
big-count-with-small-count; pair p is served by cores 2p (H-half 0) and
2p+1 (H-half 1).  Each core runs BOTH experts of its pair over the
pair's full token groups, but only its half of the hidden dimension
(H/2 = 2048), producing partial y that the host sums.  This balances
tokens across cores (~2111/core vs 2*1152 for naive expert-parallel)
while keeping weight traffic identical (each core holds half of two
experts = one expert's worth of weights).

The small gate runs host-side; the host gathers tokens per expert,
pads to the common SPMD capacities (Ca = max big-expert count, Cb =
max small count), and scatter-adds the 16 partial outputs (2 cores x
2 experts per token) back to token order.  b2 is folded on the host
(out += combine_w @ b2, exact for any b2).

Device kernel (per core), all matmuls bf16 (fp32 PSUM accumulation):
  for e in (a, b):
    mm1 (tile-major): hT[mi][:, tile] = relu(W1_mi^T @ x^T + b1)  bf16
    mm2 (token-tile): py[t] = sum_mi hT[mi][:,t]^T @ W2_mi   (one PSUM
        residency accumulates the whole H/2 contraction)
    flush: y[t] = py[t] * wc[t]  on the scalar engine (per-partition
        combine-weight scale), DMA out from SBUF on the ACT queue.
Weights stream through SBUF exactly once; W1 of the running expert is
fully resident (32 KB/partition bf16)."""

import numpy as np

P = 128
D = 1024
H = 4096
HH = H // 2       # per-core hidden half
E = 8
TOPK = 2
DK = D // P       # 8  contraction chunks for mm1
MH = HH // P      # 16 hidden chunks per core per expert


def _mm1_tiles(C):
    """Split C tokens into equal matmul moving-dim chunks (<=512 for one
    PSUM bank; chunks of ~340+ keep the bf16 matmul ahead of its 107 ns
    LDWEIGHTS so the PE is row-paced, not weight-load-paced)."""
    n = max(1, -(-C // 512))
    base, r = divmod(C, n)
    return [base + (1 if i < r else 0) for i in range(n)]


def _build_program(Ca, Cb):
    import concourse.mybir as mybir
    import concourse.tile as tile
    from concourse import bacc

    f32 = mybir.dt.float32
    bf16 = mybir.dt.bfloat16
    Relu = mybir.ActivationFunctionType.Relu
    Copy = mybir.ActivationFunctionType.Copy
    Ta = -(-Ca // P)
    Tb = -(-Cb // P)
    tts_a = _mm1_tiles(Ca)
    tts_b = _mm1_tiles(Cb)

    nc = bacc.Bacc(
        "TRN2",
        target_bir_lowering=False,
        debug=False,
        enable_asserts=True,
        num_devices=E,
    )
    # x tile-blocked: [P, sum_t DK*tsz_t] with per-tile [DK, tsz] blocks so
    # each tile loads as one contiguous multi-KB burst per partition
    xa_d = nc.dram_tensor("xa", [P, DK * Ca], bf16, kind="ExternalInput").ap()
    xb_d = nc.dram_tensor("xb", [P, DK * Cb], bf16, kind="ExternalInput").ap()
    # W1 chunk-major: [P, MH, DK, P] so each per-chunk DMA moves a
    # contiguous 2 KB per partition (full-rate DMA bursts)
    w1a_d = nc.dram_tensor("w1a", [P, MH, DK, P], bf16, kind="ExternalInput").ap()
    w1b_d = nc.dram_tensor("w1b", [P, MH, DK, P], bf16, kind="ExternalInput").ap()
    w2a_d = nc.dram_tensor("w2a", [HH, D], bf16, kind="ExternalInput").ap()
    w2b_d = nc.dram_tensor("w2b", [HH, D], bf16, kind="ExternalInput").ap()
    b1a_d = nc.dram_tensor("b1a", [P, MH], f32, kind="ExternalInput").ap()
    b1b_d = nc.dram_tensor("b1b", [P, MH], f32, kind="ExternalInput").ap()
    wca_d = nc.dram_tensor("wca", [P, Ta], f32, kind="ExternalInput").ap()
    wcb_d = nc.dram_tensor("wcb", [P, Tb], f32, kind="ExternalInput").ap()
    ya_d = nc.dram_tensor("ya", [Ca, D], f32, kind="ExternalOutput").ap()
    yb_d = nc.dram_tensor("yb", [Cb, D], f32, kind="ExternalOutput").ap()

    with tile.TileContext(nc) as tc:
        with (
            tc.tile_pool(name="const", bufs=1) as const,
            tc.tile_pool(name="w1p", bufs=2) as w1p,
            tc.tile_pool(name="w2p", bufs=24) as w2p,
            tc.tile_pool(name="htp", bufs=MH) as htp,
            tc.tile_pool(name="ysp", bufs=3) as ysp,
            tc.tile_pool(name="php", bufs=3, space="PSUM") as php,
            tc.tile_pool(name="pyp", bufs=2, space="PSUM") as pyp,
            tc.tile_pool(name="pwp", bufs=1, space="PSUM") as pwp,
        ):
            # ---- SBUF destination tiles -------------------------------
            # W1 of each expert lives fully in SBUF while its mm1 runs
            # (16 KB/partition bf16); per-m-chunk DMAs so the tensor
            # engine can chase the stream from the first chunk.
            w1t = {}
            b1a = const.tile([P, MH], f32)
            b1b = const.tile([P, MH], f32)
            wca = const.tile([P, Ta], f32)
            wcb = const.tile([P, Tb], f32)

            def load_w1(which, w1_d):
                t = w1p.tile([P, MH, DK, P], bf16, tag="w1", name=f"w1_{which}")
                for mi in range(MH):
                    nc.sync.dma_start(t[:, mi], w1_d[:, mi])
                w1t[which] = t

            def load_x_tile(x_d, which, ti, off, tsz):
                t = const.tile([P, DK, tsz], bf16, name=f"x_{which}_{ti}")
                nc.sync.dma_start(t[:], x_d[:, DK * off:DK * (off + tsz)])
                return t

            def load_w2(w2_d, mi, name):
                t = w2p.tile([P, D], bf16, tag="w2", name=name)
                nc.sync.dma_start(t[:], w2_d[mi * P:(mi + 1) * P, :])
                return t

            # ---- DMA emission order (single in-order SP queue) --------
            # expert a: x tile0, w1a chunk-by-chunk (interleaved with the
            # larger x tiles), biases, then W2a; expert b's inputs follow
            # and stream during expert a's compute.
            xa_tiles = [load_x_tile(xa_d, "a", 0, 0, tts_a[0])]
            ta = w1p.tile([P, MH, DK, P], bf16, tag="w1", name="w1_a")
            w1t["a"] = ta
            for mi in range(4):
                nc.sync.dma_start(ta[:, mi], w1a_d[:, mi])
            nc.sync.dma_start(b1a[:], b1a_d[:])
            nc.sync.dma_start(wca[:], wca_d[:])
            off = tts_a[0]
            if len(tts_a) > 1:
                xa_tiles.append(load_x_tile(xa_d, "a", 1, off, tts_a[1]))
                off += tts_a[1]
            for mi in range(4, MH):
                nc.sync.dma_start(ta[:, mi], w1a_d[:, mi])
            for ti, tsz in enumerate(tts_a[2:], start=2):
                xa_tiles.append(load_x_tile(xa_d, "a", ti, off, tsz))
                off += tsz
            w2a = [load_w2(w2a_d, mi, f"w2a_{mi}") for mi in range(MH)]
            nc.sync.dma_start(b1b[:], b1b_d[:])
            nc.sync.dma_start(wcb[:], wcb_d[:])
            xb_tiles = []
            off = 0
            for ti, tsz in enumerate(tts_b):
                xb_tiles.append(load_x_tile(xb_d, "b", ti, off, tsz))
                off += tsz
            load_w1("b", w1b_d)
            w2b = [load_w2(w2b_d, mi, f"w2b_{mi}") for mi in range(MH)]

            # ---- PE warm-up on a zeroed tile: ramps the DVFS while the
            # first operands stream in.
            warm = const.tile([P, P], bf16)
            nc.any.memset(warm[:], 0.0)
            pw = pwp.tile([P, P], f32, tag="pw")
            for _ in range(12):
                nc.tensor.matmul(pw[:], warm[:], warm[:], start=True, stop=True)

            # ---- per-expert compute -----------------------------------
            def expert(which, C, tts, T, b1t, wct, w2s, y_d):
                w1s = w1t[which]
                x_tiles = xa_tiles if which == "a" else xb_tiles
                hts = [
                    htp.tile([P, Ca], bf16, tag="ht", name=f"ht_{which}_{mi}")
                    for mi in range(MH)
                ]
                # mm1, tile-major: x tile0 + first W1 chunk suffice to start
                off = 0
                for ti, tsz in enumerate(tts):
                    xt = x_tiles[ti]
                    for mi in range(MH):
                        ph = php.tile([P, 512], f32, tag="ph")
                        for dk in range(DK):
                            nc.tensor.matmul(
                                ph[:, :tsz],
                                w1s[:, mi, dk, :],
                                xt[:, dk, :],
                                start=(dk == 0),
                                stop=(dk == DK - 1),
                            )
                        nc.scalar.activation(
                            hts[mi][:, off:off + tsz], ph[:, :tsz], Relu,
                            bias=b1t[:, mi:mi + 1],
                        )
                    off += tsz
                # mm2: whole H/2 contraction in one PSUM residency per
                # 128-token tile, then scale-by-combine-weight flush.
                for t in range(T):
                    np_ = min(P, C - t * P)
                    py = pyp.tile([P, D], f32, tag="py")
                    for mi in range(MH):
                        for h2 in range(2):
                            nc.tensor.matmul(
                                py[:np_, h2 * 512:(h2 + 1) * 512],
                                hts[mi][:, t * P:t * P + np_],
                                w2s[mi][:, h2 * 512:(h2 + 1) * 512],
                                start=(mi == 0),
                                stop=(mi == MH - 1),
                            )
                    ys = ysp.tile([P, D], f32, tag="ys")
                    last = which == "b" and t == T - 1
                    if last:
                        # final flush: scalar and vector engines each take
                        # half so the last store leaves ASAP
                        nc.scalar.activation(
                            ys[:np_, 0:512], py[:np_, 0:512],
                            Copy, scale=wct[:np_, t:t + 1],
                        )
                        nc.vector.tensor_scalar_mul(
                            ys[:np_, 512:1024], py[:np_, 512:1024],
                            wct[:np_, t:t + 1],
                        )
                        nc.sync.dma_start(
                            y_d[t * P:t * P + np_, 0:512], ys[:np_, 0:512]
                        )
                        nc.scalar.dma_start(
                            y_d[t * P:t * P + np_, 512:1024], ys[:np_, 512:1024]
                        )
                    else:
                        nc.scalar.activation(
                            ys[:np_], py[:np_], Copy, scale=wct[:np_, t:t + 1]
                        )
                        # alternate the two HWDGE queues so consecutive
                        # tile stores overlap instead of serializing
                        eng = nc.sync if t % 2 == 0 else nc.scalar
                        eng.dma_start(y_d[t * P:t * P + np_, :], ys[:np_])

            expert("a", Ca, tts_a, Ta, b1a, wca, w2a, ya_d)
            expert("b", Cb, tts_b, Tb, b1b, wcb, w2b, yb_d)
    nc.compile()
    return nc


def _route(x, Wg, bg):
    """Host gate: softmax over experts + stable top-2 (mirrors
    jax.lax.top_k tie-breaking: lowest index first)."""
    logits = x @ Wg + bg
    mx = logits.max(axis=1, keepdims=True)
    ex = np.exp(logits - mx)
    gate = ex / ex.sum(axis=1, keepdims=True)
    top2 = np.argsort(-gate, axis=1, kind="stable")[:, :TOPK]
    return gate, top2


def _pack_x(x, idx, C, bf16):
    """Gathered tokens -> [P, DK*C] bf16, tile-blocked: per mm1 tile a
    contiguous [DK, tsz] block per partition (single-burst DMA loads)."""
    xe = np.zeros((C, D), np.float32)
    xe[: len(idx)] = x[idx]
    blocks = []
    off = 0
    for tsz in _mm1_tiles(C):
        blk = xe[off:off + tsz].T.reshape(DK, P, tsz).transpose(1, 0, 2)
        blocks.append(blk.reshape(P, DK * tsz))
        off += tsz
    return np.ascontiguousarray(np.concatenate(blocks, axis=1)).astype(bf16)


def _pack_wc(gate_col, idx, C):
    """Combine weights for one expert -> [P, ceil(C/P)] (token-chunked,
    zero-padded past the token count)."""
    T = -(-C // P)
    wc = np.zeros((T * P,), np.float32)
    wc[: len(idx)] = gate_col[idx]
    return np.ascontiguousarray(wc.reshape(T, P).T)


def kernel(x, Wg, bg, W1, b1, W2, b2):
    import ml_dtypes
    from concourse.bass_utils import run_bass_kernel_spmd

    bf16 = ml_dtypes.bfloat16
    x = np.asarray(x, np.float32)
    Wg = np.asarray(Wg, np.float32)
    bg = np.asarray(bg, np.float32)
    W1 = np.asarray(W1, np.float32)
    b1 = np.asarray(b1, np.float32)
    W2 = np.asarray(W2, np.float32)
    b2 = np.asarray(b2, np.float32)
    Ttok = x.shape[0]

    gate, top2 = _route(x, Wg, bg)
    expert_idx = [
        np.nonzero((top2 == e).any(axis=1))[0] for e in range(E)
    ]
    cnts = np.array([len(s) for s in expert_idx])
    order = np.argsort(-cnts, kind="stable")
    bigs = order[:4]
    smalls = order[4:][::-1]          # pair i-th largest with i-th smallest
    # exact SPMD capacities (mm1 cost scales with C; only the wc layout
    # and mm2 tile count are 128-granular)
    Ca = max(P * 2, int(cnts[bigs].max()))
    Cb = max(P * 2, int(cnts[smalls].max()))

    nc = _build_program(Ca, Cb)

    in_maps = []
    for p in range(4):
        ea, eb = int(bigs[p]), int(smalls[p])
        ia, ib = expert_idx[ea], expert_idx[eb]
        xa = _pack_x(x, ia, Ca, bf16)
        xb = _pack_x(x, ib, Cb, bf16)
        wca = _pack_wc(gate[:, ea], ia, Ca)
        wcb = _pack_wc(gate[:, eb], ib, Cb)
        for half in range(2):
            hs = slice(half * HH, (half + 1) * HH)
            w1a = np.ascontiguousarray(
                W1[ea][:, hs].reshape(DK, P, MH, P).transpose(1, 2, 0, 3)
            ).astype(bf16)
            w1b = np.ascontiguousarray(
                W1[eb][:, hs].reshape(DK, P, MH, P).transpose(1, 2, 0, 3)
            ).astype(bf16)
            in_maps.append({
                "xa": xa, "xb": xb,
                "w1a": w1a, "w1b": w1b,
                "w2a": np.ascontiguousarray(W2[ea][hs, :]).astype(bf16),
                "w2b": np.ascontiguousarray(W2[eb][hs, :]).astype(bf16),
                "b1a": np.ascontiguousarray(b1[ea][hs].reshape(MH, P).T),
                "b1b": np.ascontiguousarray(b1[eb][hs].reshape(MH, P).T),
                "wca": wca, "wcb": wcb,
            })

    results = run_bass_kernel_spmd(nc, in_maps, core_ids=list(range(E))).results

    out = np.zeros((Ttok, D), np.float32)
    for p in range(4):
        ea, eb = int(bigs[p]), int(smalls[p])
        ia, ib = expert_idx[ea], expert_idx[eb]
        for half in range(2):
            r = results[2 * p + half]
            out[ia] += r["ya"][: len(ia)]
            out[ib] += r["yb"][: len(ib)]
    # b2 contribution, folded on the host (exact for any b2)
    mask = np.zeros((Ttok, E), np.float32)
    np.put_along_axis(mask, top2, 1.0, axis=1)
    out += (gate * mask) @ b2
    return out
